# revision 1
# baseline (speedup 1.0000x reference)
"""Bass/Tile kernel for nn_Head (softmax-first attention with post-softmax
strict-upper causal mask), SPMD over 8 TRN2 NeuronCores.

  q = x @ Wq; k = y @ Wk; v = y @ Wv        (B=4, N=M=4096, C=1024, D=128)
  a = softmax(q k^T / sqrt(D))              (full-row softmax)
  a = triu(a, k=1)                          (post-softmax mask, keeps j > i)
  out = a @ v

Sharding: core (b, h) = (core//2, core%2) handles batch b, row-blocks
2t+h (t=0..15) of 128 rows each (interleaved for AV load balance).
"""
import sys
sys.path.insert(0, '/opt/trn_rl_repo')

from contextlib import ExitStack

import numpy as np
import ml_dtypes

import concourse.bass as bass
import concourse.bacc as bacc
import concourse.tile as tile
from concourse import mybir
from concourse.bass_utils import run_bass_kernel_spmd
from concourse.masks import make_identity

F32 = mybir.dt.float32
BF16 = mybir.dt.bfloat16
NPBF16 = ml_dtypes.bfloat16

B, N, M, C, D = 4, 4096, 4096, 1024, 128
NCORES = 8
NLOC = N // 2              # 2048 rows per core
NBLK = NLOC // 128         # 16 i-blocks per core
JCH = M // 128             # 32 j-chunks
CCH = C // 128             # 8 contraction chunks
SCALE = 1.0 / np.sqrt(np.float32(D))

_CACHE = {}
TRACE = False
TRACE_DIR = "/tmp/attn_trace"


def build_nc():
    nc = bacc.Bacc("TRN2", target_bir_lowering=False, debug=False,
                   num_devices=NCORES)
    xsT_d = nc.dram_tensor("xsT", [C, NLOC], BF16, kind="ExternalInput").ap()
    yT_d = nc.dram_tensor("yT", [C, M], BF16, kind="ExternalInput").ap()
    wq_d = nc.dram_tensor("wq", [C, D], BF16, kind="ExternalInput").ap()
    wk_d = nc.dram_tensor("wk", [C, D], BF16, kind="ExternalInput").ap()
    wv_d = nc.dram_tensor("wv", [C, D], BF16, kind="ExternalInput").ap()
    dmask_d = nc.dram_tensor("dmask", [128, 256], BF16, kind="ExternalInput").ap()
    out_d = nc.dram_tensor("out", [NLOC, D], F32, kind="ExternalOutput").ap()

    copy_flip = [0]

    with tile.TileContext(nc) as tc:
        with ExitStack() as ctx:
            const = ctx.enter_context(tc.tile_pool(name="const", bufs=1))
            stage = ctx.enter_context(tc.tile_pool(name="stage", bufs=3))
            big = ctx.enter_context(tc.tile_pool(name="big", bufs=1))
            epool = ctx.enter_context(tc.tile_pool(name="epool", bufs=3))
            atsb = ctx.enter_context(tc.tile_pool(name="atsb", bufs=4))
            small = ctx.enter_context(tc.tile_pool(name="small", bufs=4))
            outp = ctx.enter_context(tc.tile_pool(name="outp", bufs=3))
            # PSUM pools: S 4 banks + at 2 + ps_small 2 = 8 banks
            spsum = ctx.enter_context(tc.tile_pool(name="spsum", bufs=2, space="PSUM"))
            atpsum = ctx.enter_context(tc.tile_pool(name="atpsum", bufs=2, space="PSUM"))
            pssm = ctx.enter_context(tc.tile_pool(name="pssm", bufs=2, space="PSUM"))


            def alt_copy(dst, srcap):
                # alternate PSUM->SBUF drains between DVE and ScalarE
                if copy_flip[0] % 2 == 0:
                    nc.vector.tensor_copy(dst, srcap)
                else:
                    nc.scalar.copy(dst, srcap)
                copy_flip[0] += 1

            # ---- constants ----
            wq_sb = const.tile([128, CCH, D], BF16)
            wk_sb = const.tile([128, CCH, D], BF16)
            wv_sb = const.tile([128, CCH, D], BF16)
            nc.sync.dma_start(out=wq_sb, in_=wq_d.rearrange("(c p) d -> p c d", p=128))
            nc.sync.dma_start(out=wk_sb, in_=wk_d.rearrange("(c p) d -> p c d", p=128))
            nc.sync.dma_start(out=wv_sb, in_=wv_d.rearrange("(c p) d -> p c d", p=128))
            dmask_sb = const.tile([128, 256], BF16)
            nc.sync.dma_start(out=dmask_sb, in_=dmask_d)
            ident = const.tile([128, 128], BF16)
            make_identity(nc, ident)

            # ---- resident tensors ----
            kT_sb = big.tile([128, M], BF16)          # k^T [d, j]
            v_sb = big.tile([128, JCH, D], BF16)      # v [j-in-chunk, chunk, d]
            qT_sb = big.tile([128, NLOC], BF16)       # q^T [d, i]

            yT_view = yT_d.rearrange("(c p) m -> p c m", p=128)
            xsT_view = xsT_d.rearrange("(c p) n -> p c n", p=128)

            # ---- phase 1: k^T and v from y ----
            for jt in range(M // 512):
                yT = stage.tile([128, CCH, 512], BF16, tag="xyT")
                nc.sync.dma_start(out=yT,
                                  in_=yT_view[:, :, jt * 512:(jt + 1) * 512])
                kT_ps = pssm.tile([128, 512], F32, tag="ps_small")
                for c in range(CCH):
                    nc.tensor.matmul(kT_ps, wk_sb[:, c, :], yT[:, c, :],
                                     start=(c == 0), stop=(c == CCH - 1))
                nc.scalar.copy(kT_sb[:, jt * 512:(jt + 1) * 512], kT_ps)
                v_ps = pssm.tile([128, 4, 128], F32, tag="ps_small")
                for b4 in range(4):
                    for c in range(CCH):
                        nc.tensor.matmul(v_ps[:, b4, :],
                                         yT[:, c, b4 * 128:(b4 + 1) * 128],
                                         wv_sb[:, c, :],
                                         start=(c == 0), stop=(c == CCH - 1))
                nc.scalar.copy(v_sb[:, 4 * jt:4 * jt + 4, :], v_ps)

            # ---- phase 1.5: q^T from xs ----
            for tau in range(NLOC // 512):
                xT = stage.tile([128, CCH, 512], BF16, tag="xyT")
                nc.sync.dma_start(out=xT,
                                  in_=xsT_view[:, :, tau * 512:(tau + 1) * 512])
                qT_ps = pssm.tile([128, 512], F32, tag="ps_small")
                for c in range(CCH):
                    nc.tensor.matmul(qT_ps, wq_sb[:, c, :], xT[:, c, :],
                                     start=(c == 0), stop=(c == CCH - 1))
                nc.scalar.copy(qT_sb[:, tau * 512:(tau + 1) * 512], qT_ps)


            # ---- phase 2: attention per i-block ----
            for t in range(NBLK):
                lhs_q = qT_sb[:, t * 128:(t + 1) * 128]
                E = epool.tile([128, M], BF16, tag="E")
                den = small.tile([128, 4], F32, tag="den")
                for quar in range(4):
                    S_ps = spsum.tile([128, 2, 512], F32, tag="S")
                    for jj in range(2):
                        jt = quar * 2 + jj
                        nc.tensor.matmul(S_ps[:, jj, :], lhs_q,
                                         kT_sb[:, jt * 512:(jt + 1) * 512],
                                         start=True, stop=True)
                    nc.scalar.activation(
                        E[:, quar * 1024:(quar + 1) * 1024],
                        S_ps.rearrange("p a b -> p (a b)"),
                        mybir.ActivationFunctionType.Exp,
                        scale=float(SCALE),
                        accum_out=den[:, quar:quar + 1])
                dsum = small.tile([128, 1], F32, tag="dsum")
                rden = small.tile([128, 1], F32, tag="rden")
                nc.vector.reduce_sum(dsum, den, axis=mybir.AxisListType.X)
                nc.vector.reciprocal(rden, dsum)
                # mask the two diagonal chunks (2t: zero/strict-upper, 2t+1)
                nc.vector.tensor_mul(E[:, 256 * t:256 * t + 256],
                                     E[:, 256 * t:256 * t + 256], dmask_sb)
                # A^T transposes + AV accumulation over kept chunks
                kept = list(range(2 * t, JCH))
                av_ps = pssm.tile([128, 128], F32, tag="ps_small")
                for g0 in range(0, len(kept), 8):
                    grp = kept[g0:g0 + 8]
                    at_ps = atpsum.tile([128, 8, 128], BF16, tag="at")
                    at_sb = atsb.tile([128, 8, 128], BF16, tag="atsb")
                    for idx, cch in enumerate(grp):
                        nc.tensor.transpose(at_ps[:, idx, :],
                                            E[:, cch * 128:(cch + 1) * 128], ident)
                    nc.vector.tensor_copy(at_sb[:, 0:len(grp), :],
                                          at_ps[:, 0:len(grp), :])
                    for idx, cch in enumerate(grp):
                        nc.tensor.matmul(av_ps, at_sb[:, idx, :], v_sb[:, cch, :],
                                         start=(cch == kept[0]),
                                         stop=(cch == kept[-1]))
                o_sb = outp.tile([128, D], F32, tag="o")
                nc.vector.tensor_scalar_mul(o_sb, av_ps, rden)
                nc.sync.dma_start(out=out_d[t * 128:(t + 1) * 128, :], in_=o_sb)

    nc.compile()
    return nc


def _get_nc():
    if "nc" not in _CACHE:
        _CACHE["nc"] = build_nc()
    return _CACHE["nc"]


def _make_dmask(h):
    m = np.zeros((128, 256), dtype=np.float32)
    upper = np.triu(np.ones((128, 128), dtype=np.float32), k=1)
    if h == 0:
        m[:, 0:128] = upper
        m[:, 128:256] = 1.0
    else:
        m[:, 0:128] = 0.0
        m[:, 128:256] = upper
    return m.astype(NPBF16)


def kernel(x, y, Wq, Wk, Wv):
    nc = _get_nc()
    xb = x.astype(NPBF16)
    yb = y.astype(NPBF16)
    wqb = Wq.astype(NPBF16)
    wkb = Wk.astype(NPBF16)
    wvb = Wv.astype(NPBF16)

    in_maps = []
    yT = {b: np.ascontiguousarray(yb[b].T) for b in range(B)}
    for core in range(NCORES):
        b, h = divmod(core, 2)
        xs = xb[b].reshape(2 * NBLK, 128, C)[h::2].reshape(NLOC, C)
        in_maps.append({
            "xsT": np.ascontiguousarray(xs.T),
            "yT": yT[b],
            "wq": wqb, "wk": wkb, "wv": wvb,
            "dmask": _make_dmask(h),
        })

    if TRACE:
        import tempfile
        tdir = tempfile.mkdtemp(prefix="attn_trace_")
        _CACHE["trace_dir"] = tdir
        res = run_bass_kernel_spmd(nc, in_maps, list(range(NCORES)),
                                   trace=True, tmpdir=tdir)
        _CACHE["exec_time_ns"] = res.exec_time_ns
    else:
        res = run_bass_kernel_spmd(nc, in_maps, list(range(NCORES)))

    out = np.empty((B, N, D), dtype=np.float32)
    for core in range(NCORES):
        b, h = divmod(core, 2)
        out[b].reshape(2 * NBLK, 128, D)[h::2] = \
            res.results[core]["out"].reshape(NBLK, 128, D)
    return out



# revision 2
# speedup vs baseline: 1.0066x; 1.0066x over previous
"""Bass/Tile kernel for nn_Head (softmax-first attention with post-softmax
strict-upper causal mask), SPMD over 8 TRN2 NeuronCores.

  q = x @ Wq; k = y @ Wk; v = y @ Wv        (B=4, N=M=4096, C=1024, D=128)
  a = softmax(q k^T / sqrt(D))              (full-row softmax)
  a = triu(a, k=1)                          (post-softmax mask, keeps j > i)
  out = a @ v

Sharding: core (b, h) = (core//2, core%2) handles batch b and the j-chunks
(128 cols each) of parity h (column split of K/V).  Each core produces a
partial numerator num^T[d, i] = sum_{own j>i} E[i,j] v[j,:] and a partial
denominator z[i] = sum_{own j} E[i,j]; the host combines
out = ((num0+num1)/(z0+z1))^T.

Scheme: scores are computed directly transposed, S^T[j, i] (kT chunk
stationary / qT streaming 512-wide), exp'd by ScalarE into E^T (bf16), the
denominator comes from DVE-folding E^T chunks 16->4 then ones-stationary
matmuls (fold-free direct matmuls for the last igroup so the tail stays
short), and AV streams E^T against a v-chunk stationary operand producing
out^T -- no PE transposes of attention tiles at all.  Emission is
slot-based so QK pairs, Z, AV and qT-projection matmuls interleave and the
PE never sits behind the (serial, ~73us) ScalarE exp stream.
"""
import sys
sys.path.insert(0, '/opt/trn_rl_repo')

from contextlib import ExitStack

import numpy as np
import ml_dtypes

import concourse.bass as bass
import concourse.bacc as bacc
import concourse.tile as tile
from concourse import mybir
from concourse.bass_utils import run_bass_kernel_spmd
from concourse.masks import make_identity

F32 = mybir.dt.float32
BF16 = mybir.dt.bfloat16
NPBF16 = ml_dtypes.bfloat16

B, N, M, C, D = 4, 4096, 4096, 1024, 128
NCORES = 8
MLOC = M // 2              # 2048 own j columns per core
LCH = MLOC // 128          # 16 own j-chunks
CCH = C // 128             # 8 contraction chunks
NG = N // 512              # 8 i-groups of 512 rows
SCALE = 1.0 / np.sqrt(np.float32(D))

_CACHE = {}
TRACE = False


def build_nc_spmd():
    nc = bacc.Bacc("TRN2", target_bir_lowering=False, debug=False,
                   num_devices=NCORES)
    xT_d = nc.dram_tensor("xT", [128, NG * CCH * 512], BF16,
                          kind="ExternalInput").ap()
    yT_d = nc.dram_tensor("yT", [128, 4 * CCH * 512], BF16,
                          kind="ExternalInput").ap()
    # packed constants: wk | wq | wv | bmask, one DMA
    cst_d = nc.dram_tensor("cst", [128, 4 * CCH * D], BF16,
                           kind="ExternalInput").ap()
    outT_d = nc.dram_tensor("outT", [D, N], F32, kind="ExternalOutput").ap()
    z_d = nc.dram_tensor("z", [1, N], F32, kind="ExternalOutput").ap()

    xT_view = xT_d.rearrange("p (t c j) -> p t c j", c=CCH, j=512)
    yT_view = yT_d.rearrange("p (t c j) -> p t c j", c=CCH, j=512)

    with tile.TileContext(nc) as tc:
        with ExitStack() as ctx:
            const = ctx.enter_context(tc.tile_pool(name="const", bufs=1))
            xstage = ctx.enter_context(tc.tile_pool(name="xstage", bufs=3))
            big = ctx.enter_context(tc.tile_pool(name="big", bufs=1))
            etp = ctx.enter_context(tc.tile_pool(name="etp", bufs=2))
            etm = ctx.enter_context(tc.tile_pool(name="etm", bufs=2))
            zf8p = ctx.enter_context(tc.tile_pool(name="zf8p", bufs=2))
            zf4p = ctx.enter_context(tc.tile_pool(name="zf4p", bufs=2))
            osb = ctx.enter_context(tc.tile_pool(name="osb", bufs=2))
            # PSUM: ST 3 bufs x 2 banks + Z 1 + AV 1 = 8 banks
            ps_st = ctx.enter_context(tc.tile_pool(name="ps_st", bufs=3,
                                                   space="PSUM"))
            ps_z = ctx.enter_context(tc.tile_pool(name="ps_z", bufs=1,
                                                  space="PSUM"))
            ps_av = ctx.enter_context(tc.tile_pool(name="ps_av", bufs=1,
                                                   space="PSUM"))

            # ---- constants in ONE DMA, then x tile 0, then yT tiles ----
            cst_sb = const.tile([128, 4, CCH, D], BF16)
            nc.sync.dma_start(out=cst_sb,
                              in_=cst_d.rearrange("p (w c d) -> p w c d",
                                                  c=CCH, d=D))
            wk_sb = cst_sb[:, 0]
            wq_sb = cst_sb[:, 1]
            wv_sb = cst_sb[:, 2]
            bmask_sb = cst_sb[:, 3].rearrange("p c d -> p (c d)").rearrange(
                "p (a b) -> p a b", b=512)
            xts = {}

            def qT_dma(it):
                xts[it] = xstage.tile([128, CCH, 512], BF16, tag="xt",
                                      name="xt")
                nc.sync.dma_start(out=xts[it], in_=xT_view[:, it])

            qT_dma(0)
            yT_sb = big.tile([128, 4, CCH, 512], BF16)
            for jt in range(4):
                nc.sync.dma_start(out=yT_sb[:, jt], in_=yT_view[:, jt])
            ident = const.tile([128, 128], BF16)
            make_identity(nc, ident)
            ones = const.tile([128, 1], BF16)
            nc.vector.memset(ones, 1.0)

            # ---- resident tensors ----
            kT_sb = big.tile([128, MLOC], BF16)        # [d, own j]
            vT_sb = big.tile([128, MLOC], BF16)        # [d, own j] staging
            v_sb = big.tile([128, LCH, D], BF16)       # [j-in-chunk, l, d]
            qT_sb = big.tile([128, N], BF16)           # [d, i]
            z_sb = big.tile([1, NG, 512], F32)         # denominators

            def kT_chain(jt):
                kp = ps_st.tile([128, 2, 512], F32, tag="st", name="kp")
                for c in range(CCH):
                    nc.tensor.matmul(kp[:, 0, :], wk_sb[:, c, :],
                                     yT_sb[:, jt, c, :],
                                     start=(c == 0), stop=(c == CCH - 1))
                nc.vector.tensor_copy(kT_sb[:, jt * 512:(jt + 1) * 512],
                                      kp[:, 0, :])

            def qT_mms(it):
                qp = ps_st.tile([128, 2, 512], F32, tag="st", name="qp")
                for c in range(CCH):
                    nc.tensor.matmul(qp[:, 0, :], wq_sb[:, c, :],
                                     xts[it][:, c, :],
                                     start=(c == 0), stop=(c == CCH - 1))
                nc.vector.tensor_copy(qT_sb[:, it * 512:(it + 1) * 512],
                                      qp[:, 0, :])

            qT_dma(1)
            qT_dma(2)
            qT_mms(0)
            kT_chain(0)

            # ---- state for the slot machine ----
            et_tiles = {}
            etm_tiles = {}
            zf4 = {}
            zps = ps_z.tile([128, 512], F32, tag="z", name="zps")
            z7st = {"ps": None, "done": 0}
            av_state = {}
            av_backlog = []            # (ready_slot, G, l)
            av_done = {G: 0 for G in range(NG)}
            vt_jobs = [(jt, c) for jt in range(4) for c in range(CCH)]
            vtps = {}

            def emit_qk_pair(G, p):
                if p == 0:
                    et_tiles[G] = etp.tile([128, LCH, 512], BF16, tag="et",
                                           name="et")
                et = et_tiles[G]
                st = ps_st.tile([128, 2, 512], F32, tag="st", name="st")
                for k in range(2):
                    l = 2 * p + k
                    nc.tensor.matmul(st[:, k, :],
                                     kT_sb[:, l * 128:(l + 1) * 128],
                                     qT_sb[:, G * 512:(G + 1) * 512],
                                     start=True, stop=True)
                nc.scalar.activation(
                    et[:, 2 * p:2 * p + 2, :].rearrange("p a b -> p (a b)"),
                    st.rearrange("p a b -> p (a b)"),
                    mybir.ActivationFunctionType.Exp,
                    scale=float(SCALE))

            def emit_mask(G):
                """Masked copies of the two band chunks into etm[G]."""
                et = et_tiles[G]
                em = etm.tile([128, 2, 512], BF16, tag="etm", name="em")
                etm_tiles[G] = em
                for i in range(2):
                    nc.vector.tensor_mul(em[:, i, :], et[:, 2 * G + i, :],
                                         bmask_sb[:, i, :])

            def emit_fold(G):
                et = et_tiles[G]
                ev = et.rearrange("p (a two) b -> p a two b", two=2)
                z8 = zf8p.tile([128, 8, 512], BF16, tag="z8", name="z8")
                nc.vector.tensor_add(z8, ev[:, :, 0, :], ev[:, :, 1, :])
                z8v = z8.rearrange("p (a two) b -> p a two b", two=2)
                z4 = zf4p.tile([128, 4, 512], BF16, tag="z4", name="z4")
                nc.vector.tensor_add(z4, z8v[:, :, 0, :], z8v[:, :, 1, :])
                zf4[G] = z4

            def emit_z7(navail):
                """Fold-free denominator for the last igroup: direct
                ones-matmuls on E^T chunks as their exps complete."""
                G = NG - 1
                while z7st["done"] < min(navail, LCH):
                    l = z7st["done"]
                    if z7st["ps"] is None:
                        z7st["ps"] = ps_av.tile([128, 512], F32, tag="av",
                                                name="z7ps")
                    nc.tensor.matmul(z7st["ps"][0:1, :], ones,
                                     et_tiles[G][:, l, :],
                                     start=(l == 0), stop=(l == LCH - 1))
                    z7st["done"] = l + 1

            def queue_av(G, slot_now):
                """Queue AV matmuls for igroup G into the backlog."""
                base = 8 * G
                # band chunks (masked copies) first; ready after mask
                for i in range(2):
                    av_backlog.append((max(base + G + 3, 9), G, 2 * G + i,
                                       ('em', i)))
                for l in range(2 * G + 2, LCH):
                    av_backlog.append((max(base + l // 2 + 3, 9), G,
                                       ('et', l)[1], ('et', l)))

            def emit_av_mm(G, l, src):
                if G not in av_state:
                    av_state[G] = ps_av.tile([128, 512], F32, tag="av",
                                             name="av")
                av = av_state[G]
                kind, idx = src
                rhs = (etm_tiles[G][:, idx, :] if kind == 'em'
                       else et_tiles[G][:, idx, :])
                nav = 16 - 2 * G
                av_done[G] += 1
                nc.tensor.matmul(av, v_sb[:, l, :], rhs,
                                 start=(av_done[G] == 1),
                                 stop=(av_done[G] == nav))
                if av_done[G] == nav:
                    ot = osb.tile([128, 512], F32, tag="ot", name="ot")
                    nc.vector.tensor_copy(ot, av)
                    nc.sync.dma_start(out=outT_d[:, G * 512:(G + 1) * 512],
                                      in_=ot)

            def drain_av(slot, budget):
                n = 0
                while n < budget and av_backlog and av_backlog[0][0] <= slot:
                    _, G, l, src = av_backlog.pop(0)
                    emit_av_mm(G, l, src)
                    n += 1

            # ---- slot machine: 64 pair-slots ----
            for t in range(8 * NG):
                G, p = divmod(t, 8)
                if p == 0:
                    queue_av(G, t)
                    if 3 <= G + 3 <= NG - 1:
                        qT_dma(G + 3)
                if G >= 1 and p == 1 and G + 2 <= NG - 1:
                    qT_mms(G + 2)
                if G == 1 and p == 3:
                    qT_mms(2)
                if G == 0 and p in (2, 4, 6):
                    kT_chain(p // 2)
                if G == 0 and p == 6:
                    qT_mms(1)
                emit_qk_pair(G, p)
                if G == 0:
                    # vT projection interleaved under igroup 0
                    if p % 2 == 0:
                        vtps[p // 2] = ps_st.tile([128, 2, 512], F32,
                                                  tag="st", name="vtps")
                    for jt, c in vt_jobs[p * 4:(p + 1) * 4]:
                        nc.tensor.matmul(vtps[jt][:, 1, :], wv_sb[:, c, :],
                                         yT_sb[:, jt, c, :],
                                         start=(c == 0), stop=(c == CCH - 1))
                    if p % 2 == 1:
                        jt = p // 2
                        nc.vector.tensor_copy(
                            vT_sb[:, jt * 512:(jt + 1) * 512],
                            vtps[jt][:, 1, :])
                    if p == 7:
                        # vT -> v transposes through PSUM
                        at = ps_st.tile([128, LCH, 128], BF16, tag="st",
                                        name="at")
                        for l in range(LCH):
                            nc.tensor.transpose(
                                at[:, l, :], vT_sb[:, l * 128:(l + 1) * 128],
                                ident)
                        nc.vector.tensor_copy(v_sb, at)
                if p == G:
                    emit_mask(G)
                if p == 7 and G < NG - 1:
                    emit_fold(G)
                # Z matmuls for the previous igroup at slots 4..7
                if G >= 1 and 4 <= p <= 7:
                    nc.tensor.matmul(zps[0:1, :], ones, zf4[G - 1][:, p - 4, :],
                                     start=(p == 4), stop=(p == 7))
                    if p == 7:
                        nc.vector.tensor_copy(z_sb[0:1, G - 1, :], zps[0:1, :])
                drain_av(t, 3 if G > 0 else 2)
                if G == NG - 1 and p >= 3:
                    emit_z7(2 * (p - 2))

            # ---- tail ----
            emit_z7(LCH)
            nc.vector.tensor_copy(z_sb[0:1, NG - 1, :], z7st["ps"][0:1, :])
            drain_av(10 ** 9, 10 ** 9)
            nc.sync.dma_start(out=z_d, in_=z_sb.rearrange("p a b -> p (a b)"))

    nc.compile()
    return nc


def _get_nc():
    if "nc" not in _CACHE:
        _CACHE["nc"] = build_nc_spmd()
    return _CACHE["nc"]


def _make_bandmask(h):
    """Masks for the two band chunks of each igroup.

    For igroup G, band chunk l=2G+i (i in 0,1) is global chunk
    c = 2l + h = 4G + 2i + h.  Against the 4 blocks g = 4G + b (b=0..3):
      b <  bdiag: keep (1.0);  b == bdiag: strict j>i;  b > bdiag: zero
    with bdiag = 2i + h (independent of G).
    """
    m = np.zeros((128, 2, 512), dtype=np.float32)
    tri = np.tril(np.ones((128, 128), dtype=np.float32), k=-1)  # j > i
    for i in range(2):
        bdiag = 2 * i + h
        for b in range(4):
            if b < bdiag:
                m[:, i, b * 128:(b + 1) * 128] = 1.0
            elif b == bdiag:
                m[:, i, b * 128:(b + 1) * 128] = tri
    return np.ascontiguousarray(m.reshape(128, 1024)).astype(NPBF16)


def _shuffle_cp(a, ntile):
    """[rows=ntile*512, C] -> [128, ntile*CCH*512] with free index (t, c, j):
    a[t*512+j, c*128+p]."""
    r = a.reshape(ntile, 512, CCH, 128).transpose(3, 0, 2, 1)
    return np.ascontiguousarray(r.reshape(128, ntile * CCH * 512))


def kernel(x, y, Wq, Wk, Wv):
    nc = _get_nc()
    xb = np.asarray(x).astype(NPBF16)
    yb = np.asarray(y).astype(NPBF16)

    def wshuf(w):
        return np.ascontiguousarray(
            np.asarray(w).astype(NPBF16).reshape(CCH, 128, D)
            .transpose(1, 0, 2).reshape(128, CCH * D))

    wqb, wkb, wvb = wshuf(Wq), wshuf(Wk), wshuf(Wv)

    in_maps = []
    xTs = {b: _shuffle_cp(xb[b], NG) for b in range(B)}
    bmasks = {h: _make_bandmask(h) for h in range(2)}
    for core in range(NCORES):
        b, h = divmod(core, 2)
        yo = yb[b].reshape(M // 128, 128, C)[h::2].reshape(MLOC, C)
        in_maps.append({
            "xT": xTs[b],
            "yT": _shuffle_cp(yo, 4),
            "cst": np.ascontiguousarray(
                np.concatenate([wkb, wqb, wvb, bmasks[h]], axis=1)),
        })

    if TRACE:
        import tempfile
        tdir = tempfile.mkdtemp(prefix="attn_trace_")
        _CACHE["trace_dir"] = tdir
        res = run_bass_kernel_spmd(nc, in_maps, list(range(NCORES)),
                                   trace=True, tmpdir=tdir)
        _CACHE["exec_time_ns"] = res.exec_time_ns
    else:
        res = run_bass_kernel_spmd(nc, in_maps, list(range(NCORES)))

    out = np.empty((B, N, D), dtype=np.float32)
    for b in range(B):
        numT = res.results[2 * b]["outT"] + res.results[2 * b + 1]["outT"]
        z = (res.results[2 * b]["z"] + res.results[2 * b + 1]["z"]).reshape(N)
        out[b] = (numT / z[None, :]).T
    return out
